# revision 44
# baseline (speedup 1.0000x reference)
"""Trainium2 Bass kernel for nn_BinaryTreeShInvariantConv.

Per (b, v): gather P=32 neighbor rows of signal[b] (Cin=64), contract over P
against conv_kernel[b,v] -> y[Cin, R*N], square, sum SH orders per degree l,
sqrt(+eps), contract [Cin*R*(L+1)=512] against kernel_weights -> [Cout=128],
bias + relu.

Sharding: data-parallel over batch B=8 -> one batch per NeuronCore (SPMD).

Design (driven by the CoreSim v1 cost model, which prices each instruction
as free-size x engine-cycle charged serially to its issuing engine):
  - Gather reads bf16 rows PACKED AS uint64 (16 u64 = 64 bf16 channels):
    the gather is priced as a generic Pool op at out-free-ELEMENTS x 0.83ns,
    so 8x fewer elements -> 4 ops x 427ns per 512-v supergroup (13.7us total
    vs 218us naive).
  - "Pair-diagonal" lhsT: gathered rows land directly in block-diagonal
    [64 part, 128 col] bf16 slabs (2 v's per slab; off-diag zeros memset
    once per buffer). MM1 -> [128 part = (v-parity, c), 32 rn] per pair:
    half the PE columns of a 4-v block-diag rhs; conv_kernel needs one
    [128, 4096] DMA per supergroup.
  - Degree sums as strided bf16 tensor_tensor adds (2x DVE mode) instead of
    reduce_sum (no fast mode).
  - Squares (PSUM f32 -> bf16) split between ACT (activation Square) and
    Pool (tensor_tensor mult) to balance engine occupancy.
  - MM3 packs 128 v-pairs in output partitions.

Supergroup = 512 v's (NSG=8): 4 gather ops (one per lhsT quadrant class,
4096 idxs each), 1 kre DMA, 8x32 MM1 matmuls -> 8 PSUM granules -> squares
-> per 256-v block: DVE degree-adds, ACT sqrt(+eps), MM3 (2 par x 8 rl
accumulating), DVE relu, store.
"""

import sys

sys.path.insert(0, "/opt/trn_rl_repo")

import numpy as np

import concourse.bacc as bacc
import concourse.mybir as mybir
import concourse.tile as tile
from concourse import ap_utils
from concourse.bass import MemorySpace
from concourse import bass2jax

B, V, P, CIN, R, COUT = 8, 4096, 32, 64, 2, 128
NSH, NDEG = 16, 4
VSG = 512            # v's per supergroup
NSG = V // VSG       # 8 supergroups
NSLAB = 128          # v-pairs per part-block (h) per supergroup
# engine per square granule (gidx = 4h+gl): balance ACT/Pool/DVE
SQ_ENG = [list("PADPAPAP"), list("PADPAPAP")]
BF16 = mybir.dt.bfloat16
F32 = mybir.dt.float32
I16 = mybir.dt.int16
U64 = mybir.dt.uint64

_CACHE = {}
_SKIP = set()  # debug: subset of {'gather','mm1','post','mm3','store'}


def _dma_gather_any(eng, out_ap, in_ap, idxs_ap, num_idxs, elem_size,
                    single_packet=True, nreg=None):
    """bass.dma_gather with relaxed asserts. The executor flattens the out AP
    and reshapes it to (128, ceil(n/128), elem) as a VIEW: row i lands at
    flat slot (i%128)*chunks + i//128. The AP's stride tree must therefore be
    numpy-reshape-compatible with that shape (seamless merges)."""
    assert idxs_ap.dtype == I16
    assert in_ap.space == MemorySpace.DRAM
    assert in_ap.dtype == out_ap.dtype
    elem_step = in_ap.ap[0][0]
    stride_bytes = elem_step * mybir.dt.size(in_ap.dtype)
    assert stride_bytes % 256 == 0 and stride_bytes // 256 < 256
    assert ap_utils.ap_is_contiguous(idxs_ap.ap[1:])
    assert in_ap.ap[-1][1] == elem_size
    total = 1
    for _, c in out_ap.ap:
        total *= c
    assert total == ((num_idxs + 127) // 128) * 128 * elem_size

    _in_ap = eng.lower_ap_dma(in_ap, for_custom_bir_dma=True)
    if nreg is None:
        nreg = eng.to_reg(num_idxs)
    return eng.add_instruction(
        mybir.InstDMAGatherAnt(
            name=eng.bass.get_next_instruction_name(),
            ins=[*_in_ap, eng.lower_ap(idxs_ap),
                 eng.lower_val_access(nreg)],
            outs=[eng.lower_ap(out_ap)],
            transpose=False,
            num_idxs=num_idxs,
            elem_size=elem_size,
            stride_bytes_256=stride_bytes // 256,
            gen_mode=0,
            single_packet=single_packet,
            queue_num=0,
            sbuf_tokens_per_rank=0,
            sbuf_free_dim_per_rank=0,
            sbuf_free_dim_pad_per_rank=0,
            sbuf_byte_offset=0,
        ))


def _build_nc(with_bias):
    nc = bacc.Bacc("TRN2", target_bir_lowering=False, debug=False,
                   enable_asserts=False, dynamic_dma_scratch_size=49152)
    AF = mybir.ActivationFunctionType
    ALU = mybir.AluOpType

    sig = nc.dram_tensor("sig", [V, 32], U64, kind="ExternalInput")
    kre = nc.dram_tensor("kre", [NSG, 128, NSLAB, 2 * NSH], BF16,
                         kind="ExternalInput")
    idx = nc.dram_tensor("idx", [128, V * P // 16], I16, kind="ExternalInput")
    wsb = nc.dram_tensor("wsb", [128, 8 * COUT], BF16, kind="ExternalInput")
    bia = nc.dram_tensor("bia", [1, COUT], F32, kind="ExternalInput")
    outd = nc.dram_tensor("outd", [V, COUT], F32, kind="ExternalOutput")

    with tile.TileContext(nc) as tc:
        with (
            tc.tile_pool(name="const", bufs=1) as constp,
            tc.tile_pool(name="patbf", bufs=2) as patbfp,
            tc.tile_pool(name="kre", bufs=3) as krep,
            tc.tile_pool(name="ysq", bufs=4) as ysqp,
            tc.tile_pool(name="zsb", bufs=2) as zsbp,
            tc.tile_pool(name="zt", bufs=2) as ztp,
            tc.tile_pool(name="osb", bufs=2) as osbp,
            tc.tile_pool(name="ps1", bufs=3, space="PSUM") as ps1p,
            tc.tile_pool(name="ps3", bufs=2, space="PSUM") as ps3p,
        ):
            w_t = constp.tile([128, 8 * COUT], BF16, tag="w")
            idx_t = constp.tile([128, V * P // 16], I16, tag="idx")
            ISG = (V * P // 16) // NSG
            if with_bias:
                bias_t = constp.tile([1, COUT], F32, tag="bias")
                nc.sync.dma_start(bias_t[:], bia.ap())
            eps_t = constp.tile([128, 1], F32, tag="eps")
            nc.vector.memset(eps_t[:], 1e-4)

            # dense patch tiles: fully rewritten by the gather each
            # supergroup, so no zeroing needed. Row (part, chunk) holds the
            # patch row of (v, p) = (sgb + 2*(4*(chunk//2) + part//32)
            # + chunk%2, part%32).
            patbfs = [patbfp.tile([128, NSLAB * 64], BF16, tag="patbf",
                                  name=f"patbf_{i}") for i in range(2)]

            nreg = nc.gpsimd.to_reg(VSG * P)

            for sg in range(NSG):
                sgb = VSG * sg
                # --- idx slice + gather: one dense 128-part op -------------
                col0 = ISG * sg
                nc.sync.dma_start(idx_t[:, col0:col0 + ISG],
                                  idx.ap()[:, col0:col0 + ISG])
                patbf = patbfs[sg % 2]
                gv = patbf[:, :].bitcast(U64).rearrange(
                    "p (ch c) -> p ch c", ch=NSLAB)
                if 'gather' not in _SKIP:
                    _dma_gather_any(
                        nc.gpsimd, gv, sig.ap()[:, 0:CIN // 4],
                        idx_t[:, col0:col0 + ISG], VSG * P,
                        CIN // 4, single_packet=False, nreg=nreg)

                # --- kre load ----------------------------------------------
                kre_t = krep.tile([128, NSLAB * 2 * NSH], BF16, tag="kre")
                nc.sync.dma_start(kre_t[:], kre.ap()[sg])
                if sg == 0:
                    # w needed only by MM3; load after the critical kre0
                    nc.sync.dma_start(w_t[:], wsb.ap())

                # --- MM1 for both part-blocks first (keeps PE dense), then
                # the post chains (adds/sqrt/MM3) so MM3(h0) never head-of-
                # line blocks MM1(h1) on the in-order PE queue.
                ysqs = []
                for h in range(2):
                    ysq = ysqp.tile([128, 4 * 32 * 32], BF16, tag="ysq")
                    ysqs.append(ysq)
                    for gl in range(4):
                        ps1 = ps1p.tile([128, 1024], F32, tag="ps1")
                        for ql in range(32 if 'mm1' not in _SKIP else 0):
                            q = 128 * h + 32 * gl + ql
                            j, t = q % 4, q // 4
                            # y_A||garbage: pat(vA|vB) x K_A -> [128, 32]
                            nc.tensor.matmul(
                                ps1[:, 32 * ql:32 * ql + 32],
                                patbf[32 * j:32 * j + 32,
                                      128 * t:128 * t + 128],
                                kre_t[32 * j:32 * j + 32,
                                      64 * t:64 * t + 32],
                                start=True, stop=False,
                                skip_group_check=True,
                                tile_position=(32 * j, 0))
                            # fix upper half: += pat(vB) x (K_B - K_A)
                            nc.tensor.matmul(
                                ps1[64:128, 32 * ql:32 * ql + 32],
                                patbf[32 * j:32 * j + 32,
                                      128 * t + 64:128 * t + 128],
                                kre_t[32 * j:32 * j + 32,
                                      64 * t + 32:64 * t + 64],
                                start=False, stop=True,
                                skip_group_check=True,
                                tile_position=(32 * j, 64))
                        if 'post' not in _SKIP:
                            dst = ysq[:, 1024 * gl:1024 * (gl + 1)]
                            se = SQ_ENG[sg % 2][4 * h + gl]
                            if se == 'A':
                                nc.scalar.activation(dst, ps1[:], AF.Square)
                            else:
                                e = nc.gpsimd if se == 'P' else nc.vector
                                e.tensor_tensor(dst, ps1[:], ps1[:], ALU.mult)

                for h in range(2):
                    ysq = ysqs[h]
                    # ysq: [128, (g 4, pl 32, n 16, r 2)]
                    yv = ysq[:, :].rearrange("p (g pl n r) -> p g pl n r",
                                             g=4, pl=32, n=NSH)
                    # n split (n2, two): n = 2*n2 + two, for stride-2 windows
                    yv2 = ysq[:, :].rearrange(
                        "p (g pl n2 two r) -> p g pl n2 two r",
                        g=4, pl=32, n2=NSH // 2, two=2)
                    # zsb: [128, (g 4, pl 32, l 4, r 2)]
                    zsb = zsbp.tile([128, 4 * 32 * NDEG * 2], BF16, tag="zsb")
                    zv = zsb[:, :].rearrange("p (g pl l r) -> p g pl l r",
                                             g=4, pl=32, l=NDEG)
                    # zt scratch: [128, (g 4, pl 32, t 5, r 2)]
                    zt = ztp.tile([128, 4 * 32 * 5 * 2], BF16, tag="zt")
                    tv = zt[:, :].rearrange("p (g pl t r) -> p g pl t r",
                                            g=4, pl=32, t=5)
                    TT = nc.vector.tensor_tensor
                    if 'post' not in _SKIP:
                        # l=3: n 9..15 (7 terms): {9,11,13}+{10,12,14}, +15
                        TT(tv[:, :, :, 0:3, :], yv2[:, :, :, 4:7, 1, :],
                           yv2[:, :, :, 5:8, 0, :], ALU.add)
                        TT(zv[:, :, :, 3, :], tv[:, :, :, 0, :],
                           tv[:, :, :, 1, :], ALU.add)
                        TT(zv[:, :, :, 3, :], zv[:, :, :, 3, :],
                           tv[:, :, :, 2, :], ALU.add)
                        TT(zv[:, :, :, 3, :], zv[:, :, :, 3, :],
                           yv2[:, :, :, 7, 1, :], ALU.add)
                        # l=2: n 4..8 (5 terms): {4,6}+{5,7}, +8
                        TT(tv[:, :, :, 3:5, :], yv2[:, :, :, 2:4, 0, :],
                           yv2[:, :, :, 2:4, 1, :], ALU.add)
                        TT(zv[:, :, :, 2, :], tv[:, :, :, 3, :],
                           tv[:, :, :, 4, :], ALU.add)
                        TT(zv[:, :, :, 2, :], zv[:, :, :, 2, :],
                           yv2[:, :, :, 4, 0, :], ALU.add)
                        # l=1: n 1..3
                        TT(zv[:, :, :, 1, :], yv2[:, :, :, 0, 1, :],
                           yv2[:, :, :, 1, 0, :], ALU.add)
                        TT(zv[:, :, :, 1, :], zv[:, :, :, 1, :],
                           yv2[:, :, :, 1, 1, :], ALU.add)
                        # sqrt(x + eps): l=0 from ysq n=0; l>=1 in place
                        nc.scalar.activation(zv[:, :, :, 0, :],
                                             yv[:, :, :, 0, :], AF.Sqrt,
                                             bias=eps_t[:])
                        nc.scalar.activation(zv[:, :, :, 1:4, :],
                                             zv[:, :, :, 1:4, :], AF.Sqrt,
                                             bias=eps_t[:])

                    # --- MM3: out[pair, (par, i)] --------------------------
                    ps3 = ps3p.tile([128, 2 * COUT], F32, tag="ps3")
                    for par in range(2 if 'mm3' not in _SKIP else 0):
                        for rl in range(8):
                            lhsT = zv[64 * par:64 * par + 64, :, :, rl // 2,
                                      rl % 2]
                            rhs = w_t[64 * par:64 * par + 64,
                                      COUT * rl:COUT * (rl + 1)]
                            nc.tensor.matmul(
                                ps3[:, COUT * par:COUT * (par + 1)],
                                lhsT, rhs, start=(rl == 0), stop=(rl == 7),
                                skip_group_check=True)

                    # --- relu (+bias) + store ------------------------------
                    osb = osbp.tile([128, 2 * COUT], F32, tag="osb")
                    if with_bias:
                        for par in range(2):
                            nc.vector.tensor_add(
                                osb[:, COUT * par:COUT * (par + 1)],
                                ps3[:, COUT * par:COUT * (par + 1)],
                                bias_t[:, :].broadcast(0, 128))
                        nc.scalar.activation(osb[:], osb[:], AF.Relu)
                    else:
                        nc.gpsimd.tensor_scalar_max(osb[:], ps3[:], 0.0)
                    if 'store' not in _SKIP:
                        dst = outd.ap()[sgb + 256 * h:sgb + 256 * (h + 1), :]
                        dst = dst.rearrange("(pl par) i -> pl par i", par=2)
                        nc.sync.dma_start(
                            dst, osb[:, :].rearrange("p (par i) -> p par i",
                                                     par=2))

    nc.compile()
    return nc


# gather row i -> (v-offset-in-sg, p): part = i%128, chunk = i//128,
# pair q = 4*(chunk//2) + part//32, v = 2q + chunk%2, p = part%32
_II = np.arange(VSG * P)
_VOFF = 2 * (4 * ((_II // 128) // 2) + (_II % 128) // 32) + (_II // 128) % 2
_POFF = (_II % 128) % 32


def _prep_inputs_core(b, signal, patches_idx, conv_kernel, kernel_weights,
                      biases):
    bf = mybir.dt.np(BF16)
    sigrow = np.zeros((V, 128), dtype=bf)
    sigrow[:, 0:CIN] = signal[b].astype(bf)
    sig_u64 = np.ascontiguousarray(sigrow).view(np.uint64)   # [V, 32]

    pidx = patches_idx[b, :, :, 1]
    krn = conv_kernel[b].transpose(0, 1, 3, 2).reshape(V, P, 2 * NSH)
    # kre[sg, 32j+p, 64t + 32b + nr]: b=0: K[vA], b=1: K[vB]-K[vA],
    # with q = 4t+j, vA = sg*512 + 2q, vB = vA + 1
    ka = krn[0::2].reshape(NSG, NSLAB * 2, P, 2 * NSH)       # [sg, q, p, nr]
    kd = (krn[1::2] - krn[0::2]).reshape(NSG, NSLAB * 2, P, 2 * NSH)
    kab = np.stack([ka, kd], axis=3)          # [sg, q, p, b, nr]
    k6 = kab.reshape(NSG, 64, 4, P, 2, 2 * NSH)   # [sg, t, j, p, b, nr]
    kre_ = np.ascontiguousarray(k6.transpose(0, 2, 3, 1, 4, 5)).reshape(
        NSG, 128, NSLAB, 2 * NSH).astype(bf)

    # idx: one op per sg, 16384 idxs; arr[i] = pidx[v(i), p(i)]
    # (rows 16..127 are zero padding -- the gather ucode reads rows 0:16;
    # loading a full-height tensor avoids an uninitialized-SBUF memset)
    idxh = np.zeros((128, V * P // 16), dtype=np.int16)
    ncols = (V * P // 16) // NSG
    for sg in range(NSG):
        arr = pidx[VSG * sg + _VOFF, _POFF].astype(np.int16)
        idxh[:16, ncols * sg:ncols * (sg + 1)] = arr.reshape(ncols, 16).T

    w = kernel_weights.transpose(1, 3, 2, 0).reshape(CIN, 8 * COUT)
    wsb = np.concatenate([w, w], axis=0).astype(bf)
    bia = biases.reshape(1, COUT).astype(np.float32)
    return {"sig": sig_u64, "kre": kre_, "idx": idxh, "wsb": wsb, "bia": bia}


def _make_runner(nc, n_cores=8):
    import jax
    from jax.sharding import Mesh, PartitionSpec
    from jax.experimental.shard_map import shard_map

    bass2jax.install_neuronx_cc_hook()
    partition_name = (nc.partition_id_tensor.name
                      if nc.partition_id_tensor else None)
    in_names, out_names, out_avals, zero_outs = [], [], [], []
    for alloc in nc.m.functions[0].allocations:
        if not isinstance(alloc, mybir.MemoryLocationSet):
            continue
        name = alloc.memorylocations[0].name
        if alloc.kind == "ExternalInput":
            if name != partition_name:
                in_names.append(name)
        elif alloc.kind == "ExternalOutput":
            out_names.append(name)
            shape = tuple(alloc.tensor_shape)
            dtype = mybir.dt.np(alloc.dtype)
            out_avals.append(jax.core.ShapedArray(shape, dtype))
            zero_outs.append(np.zeros(shape, dtype))
    n_params, n_outs = len(in_names), len(out_avals)
    in_names_all = list(in_names) + list(out_names)
    if partition_name is not None:
        in_names_all.append(partition_name)

    def _body(*args):
        operands = list(args)
        if partition_name is not None:
            operands.append(bass2jax.partition_id_tensor())
        outs = bass2jax._bass_exec_p.bind(
            *operands, out_avals=tuple(out_avals),
            in_names=tuple(in_names_all), out_names=tuple(out_names),
            lowering_input_output_aliases=(),
            sim_require_finite=True, sim_require_nnan=True, nc=nc)
        return tuple(outs)

    donate = tuple(range(n_params, n_params + n_outs))
    devices = jax.devices()[:n_cores]
    mesh = Mesh(np.asarray(devices), ("core",))
    sharded = jax.jit(
        shard_map(_body, mesh=mesh,
                  in_specs=(PartitionSpec("core"),) * (n_params + n_outs),
                  out_specs=(PartitionSpec("core"),) * n_outs,
                  check_rep=False),
        donate_argnums=donate, keep_unused=True)

    def run_fn(in_maps):
        import jax
        per_core = [[np.asarray(m[nm]) for nm in in_names] for m in in_maps]
        concat_in = [
            np.concatenate([per_core[c][i] for c in range(n_cores)], axis=0)
            for i in range(n_params)]
        concat_zeros = [
            np.zeros((n_cores * z.shape[0], *z.shape[1:]), z.dtype)
            for z in zero_outs]
        out_arrs = sharded(*concat_in, *concat_zeros)
        jax.block_until_ready(out_arrs)
        return [
            {nm: np.asarray(out_arrs[i]).reshape(n_cores, *out_avals[i].shape)[c]
             for i, nm in enumerate(out_names)}
            for c in range(n_cores)]

    return run_fn


def kernel(signal, patches_idx, conv_kernel, kernel_weights, biases):
    with_bias = bool(np.any(biases))
    key = ("k", with_bias)
    if key not in _CACHE:
        nc = _build_nc(with_bias)
        _CACHE[key] = (nc, _make_runner(nc))
    nc, run = _CACHE[key]

    in_maps = []
    for b in range(B):
        m = _prep_inputs_core(b, signal, patches_idx, conv_kernel,
                              kernel_weights, biases)
        in_maps.append(m)

    results = run(in_maps)
    out = np.stack([results[b]["outd"] for b in range(B)], axis=0)
    return out.astype(np.float32)


# revision 49
# speedup vs baseline: 1.0294x; 1.0294x over previous
"""Trainium2 Bass kernel for nn_BinaryTreeShInvariantConv.

Per (b, v): gather P=32 neighbor rows of signal[b] (Cin=64), contract over P
against conv_kernel[b,v] -> y[Cin, R*N], square, sum SH orders per degree l,
sqrt(+eps), contract [Cin*R*(L+1)=512] against kernel_weights -> [Cout=128],
bias + relu.

Sharding: data-parallel over batch B=8 -> one batch per NeuronCore (SPMD).

Design (driven by the CoreSim v1 cost model, which prices each instruction
as free-size x engine-cycle charged serially to its issuing engine):
  - Gather reads bf16 rows PACKED AS uint64 (16 u64 = 64 bf16 channels):
    the gather is priced as a generic Pool op at out-free-ELEMENTS x 0.83ns,
    so 8x fewer elements -> 4 ops x 427ns per 512-v supergroup (13.7us total
    vs 218us naive).
  - "Pair-diagonal" lhsT: gathered rows land directly in block-diagonal
    [64 part, 128 col] bf16 slabs (2 v's per slab; off-diag zeros memset
    once per buffer). MM1 -> [128 part = (v-parity, c), 32 rn] per pair:
    half the PE columns of a 4-v block-diag rhs; conv_kernel needs one
    [128, 4096] DMA per supergroup.
  - Degree sums as strided bf16 tensor_tensor adds (2x DVE mode) instead of
    reduce_sum (no fast mode).
  - Squares (PSUM f32 -> bf16) split between ACT (activation Square) and
    Pool (tensor_tensor mult) to balance engine occupancy.
  - MM3 packs 128 v-pairs in output partitions.

Supergroup = 512 v's (NSG=8): 4 gather ops (one per lhsT quadrant class,
4096 idxs each), 1 kre DMA, 8x32 MM1 matmuls -> 8 PSUM granules -> squares
-> per 256-v block: DVE degree-adds, ACT sqrt(+eps), MM3 (2 par x 8 rl
accumulating), DVE relu, store.
"""

import sys

sys.path.insert(0, "/opt/trn_rl_repo")

import numpy as np

import concourse.bacc as bacc
import concourse.mybir as mybir
import concourse.tile as tile
from concourse import ap_utils
from concourse.bass import MemorySpace
from concourse import bass2jax

B, V, P, CIN, R, COUT = 8, 4096, 32, 64, 2, 128
NSH, NDEG = 16, 4
VSG = 512            # v's per supergroup
NSG = V // VSG       # 8 supergroups
NSLAB = 128          # v-pairs per part-block (h) per supergroup
# engine per square granule (gidx = 4h+gl): balance ACT/Pool/DVE
SQ_ENG = [list("PADPAPAP"), list("PADPAPAP")]
BF16 = mybir.dt.bfloat16
F32 = mybir.dt.float32
I16 = mybir.dt.int16
U64 = mybir.dt.uint64

_CACHE = {}
_SKIP = set()  # debug: subset of {'gather','mm1','post','mm3','store'}


def _dma_gather_any(eng, out_ap, in_ap, idxs_ap, num_idxs, elem_size,
                    single_packet=True, nreg=None):
    """bass.dma_gather with relaxed asserts. The executor flattens the out AP
    and reshapes it to (128, ceil(n/128), elem) as a VIEW: row i lands at
    flat slot (i%128)*chunks + i//128. The AP's stride tree must therefore be
    numpy-reshape-compatible with that shape (seamless merges)."""
    assert idxs_ap.dtype == I16
    assert in_ap.space == MemorySpace.DRAM
    assert in_ap.dtype == out_ap.dtype
    elem_step = in_ap.ap[0][0]
    stride_bytes = elem_step * mybir.dt.size(in_ap.dtype)
    assert stride_bytes % 256 == 0 and stride_bytes // 256 < 256
    assert ap_utils.ap_is_contiguous(idxs_ap.ap[1:])
    assert in_ap.ap[-1][1] == elem_size
    total = 1
    for _, c in out_ap.ap:
        total *= c
    assert total == ((num_idxs + 127) // 128) * 128 * elem_size

    _in_ap = eng.lower_ap_dma(in_ap, for_custom_bir_dma=True)
    if nreg is None:
        nreg = eng.to_reg(num_idxs)
    return eng.add_instruction(
        mybir.InstDMAGatherAnt(
            name=eng.bass.get_next_instruction_name(),
            ins=[*_in_ap, eng.lower_ap(idxs_ap),
                 eng.lower_val_access(nreg)],
            outs=[eng.lower_ap(out_ap)],
            transpose=False,
            num_idxs=num_idxs,
            elem_size=elem_size,
            stride_bytes_256=stride_bytes // 256,
            gen_mode=0,
            single_packet=single_packet,
            queue_num=0,
            sbuf_tokens_per_rank=0,
            sbuf_free_dim_per_rank=0,
            sbuf_free_dim_pad_per_rank=0,
            sbuf_byte_offset=0,
        ))


def _build_nc(with_bias):
    nc = bacc.Bacc("TRN2", target_bir_lowering=False, debug=False,
                   enable_asserts=False, dynamic_dma_scratch_size=49152)
    AF = mybir.ActivationFunctionType
    ALU = mybir.AluOpType

    sig = nc.dram_tensor("sig", [V, 32], U64, kind="ExternalInput")
    kre = nc.dram_tensor("kre", [NSG, 128, NSLAB, 2 * NSH], BF16,
                         kind="ExternalInput")
    idx = nc.dram_tensor("idx", [128, V * P // 16], I16, kind="ExternalInput")
    wsb = nc.dram_tensor("wsb", [128, 8 * COUT], BF16, kind="ExternalInput")
    bia = nc.dram_tensor("bia", [1, COUT], F32, kind="ExternalInput")
    outd = nc.dram_tensor("outd", [V, COUT], F32, kind="ExternalOutput")

    with tile.TileContext(nc) as tc:
        with (
            tc.tile_pool(name="const", bufs=1) as constp,
            tc.tile_pool(name="patbf", bufs=2) as patbfp,
            tc.tile_pool(name="kre", bufs=3) as krep,
            tc.tile_pool(name="ysq", bufs=4) as ysqp,
            tc.tile_pool(name="zsb", bufs=2) as zsbp,
            tc.tile_pool(name="zt", bufs=2) as ztp,
            tc.tile_pool(name="osb", bufs=2) as osbp,
            tc.tile_pool(name="ps1", bufs=6, space="PSUM") as ps1p,
            tc.tile_pool(name="ps3", bufs=2, space="PSUM") as ps3p,
        ):
            w_t = constp.tile([128, 8 * COUT], BF16, tag="w")
            idx_t = constp.tile([128, V * P // 16], I16, tag="idx")
            ISG = (V * P // 16) // NSG
            if with_bias:
                bias_t = constp.tile([1, COUT], F32, tag="bias")
                nc.sync.dma_start(bias_t[:], bia.ap())
            eps_t = constp.tile([128, 1], F32, tag="eps")
            nc.vector.memset(eps_t[:], 1e-4)

            # dense patch tiles: fully rewritten by the gather each
            # supergroup, so no zeroing needed. Row (part, chunk) holds the
            # patch row of (v, p) = (sgb + 2*(4*(chunk//2) + part//32)
            # + chunk%2, part%32).
            patbfs = [patbfp.tile([128, NSLAB * 64], BF16, tag="patbf",
                                  name=f"patbf_{i}") for i in range(2)]

            nreg = nc.gpsimd.to_reg(VSG * P)

            for sg in range(NSG):
                sgb = VSG * sg
                # --- idx slice + gather: one dense 128-part op -------------
                col0 = ISG * sg
                nc.scalar.dma_start(idx_t[:, col0:col0 + ISG],
                                    idx.ap()[:, col0:col0 + ISG])
                patbf = patbfs[sg % 2]
                gv = patbf[:, :].bitcast(U64).rearrange(
                    "p (ch c) -> p ch c", ch=NSLAB)
                if 'gather' not in _SKIP:
                    _dma_gather_any(
                        nc.gpsimd, gv, sig.ap()[:, 0:CIN // 4],
                        idx_t[:, col0:col0 + ISG], VSG * P,
                        CIN // 4, single_packet=False, nreg=nreg)

                # --- kre load ----------------------------------------------
                kre_t = krep.tile([128, NSLAB * 2 * NSH], BF16, tag="kre")
                nc.sync.dma_start(kre_t[:], kre.ap()[sg])
                if sg == 0:
                    # w needed only by MM3; load after the critical kre0
                    nc.sync.dma_start(w_t[:], wsb.ap())

                # --- MM1 for both part-blocks first (keeps PE dense), then
                # the post chains (adds/sqrt/MM3) so MM3(h0) never head-of-
                # line blocks MM1(h1) on the in-order PE queue.
                ysqs = []
                for h in range(2):
                    ysq = ysqp.tile([128, 4 * 32 * 32], BF16, tag="ysq")
                    ysqs.append(ysq)
                    for gl in range(8):
                        ps1 = ps1p.tile([128, 512], F32, tag="ps1")
                        for ql in range(16 if 'mm1' not in _SKIP else 0):
                            q = 128 * h + 16 * gl + ql
                            j, t = q % 4, q // 4
                            # y_A||garbage: pat(vA|vB) x K_A -> [128, 32]
                            nc.tensor.matmul(
                                ps1[:, 32 * ql:32 * ql + 32],
                                patbf[32 * j:32 * j + 32,
                                      128 * t:128 * t + 128],
                                kre_t[32 * j:32 * j + 32,
                                      64 * t:64 * t + 32],
                                start=True, stop=False,
                                skip_group_check=True,
                                tile_position=(32 * j, 0))
                            # fix upper half: += pat(vB) x (K_B - K_A)
                            nc.tensor.matmul(
                                ps1[64:128, 32 * ql:32 * ql + 32],
                                patbf[32 * j:32 * j + 32,
                                      128 * t + 64:128 * t + 128],
                                kre_t[32 * j:32 * j + 32,
                                      64 * t + 32:64 * t + 64],
                                start=False, stop=True,
                                skip_group_check=True,
                                tile_position=(32 * j, 64))
                        if 'post' not in _SKIP:
                            dst = ysq[:, 512 * gl:512 * (gl + 1)]
                            se = SQ_ENG[sg % 2][(8 * h + gl) % 8]
                            if se == 'A':
                                nc.scalar.activation(dst, ps1[:], AF.Square)
                            else:
                                e = nc.gpsimd if se == 'P' else nc.vector
                                e.tensor_tensor(dst, ps1[:], ps1[:], ALU.mult)

                for h in range(2):
                    ysq = ysqs[h]
                    # ysq: [128, (g 4, pl 32, n 16, r 2)]
                    yv = ysq[:, :].rearrange("p (g pl n r) -> p g pl n r",
                                             g=4, pl=32, n=NSH)
                    # n split (n2, two): n = 2*n2 + two, for stride-2 windows
                    yv2 = ysq[:, :].rearrange(
                        "p (g pl n2 two r) -> p g pl n2 two r",
                        g=4, pl=32, n2=NSH // 2, two=2)
                    # zsb: [128, (g 4, pl 32, l 4, r 2)]
                    zsb = zsbp.tile([128, 4 * 32 * NDEG * 2], BF16, tag="zsb")
                    zv = zsb[:, :].rearrange("p (g pl l r) -> p g pl l r",
                                             g=4, pl=32, l=NDEG)
                    # zt scratch: [128, (g 4, pl 32, t 5, r 2)]
                    zt = ztp.tile([128, 4 * 32 * 5 * 2], BF16, tag="zt")
                    tv = zt[:, :].rearrange("p (g pl t r) -> p g pl t r",
                                            g=4, pl=32, t=5)
                    TT = nc.vector.tensor_tensor
                    if 'post' not in _SKIP:
                        # l=3: n 9..15 (7 terms): {9,11,13}+{10,12,14}, +15
                        TT(tv[:, :, :, 0:3, :], yv2[:, :, :, 4:7, 1, :],
                           yv2[:, :, :, 5:8, 0, :], ALU.add)
                        TT(zv[:, :, :, 3, :], tv[:, :, :, 0, :],
                           tv[:, :, :, 1, :], ALU.add)
                        TT(zv[:, :, :, 3, :], zv[:, :, :, 3, :],
                           tv[:, :, :, 2, :], ALU.add)
                        TT(zv[:, :, :, 3, :], zv[:, :, :, 3, :],
                           yv2[:, :, :, 7, 1, :], ALU.add)
                        # l=2: n 4..8 (5 terms): {4,6}+{5,7}, +8
                        TT(tv[:, :, :, 3:5, :], yv2[:, :, :, 2:4, 0, :],
                           yv2[:, :, :, 2:4, 1, :], ALU.add)
                        TT(zv[:, :, :, 2, :], tv[:, :, :, 3, :],
                           tv[:, :, :, 4, :], ALU.add)
                        TT(zv[:, :, :, 2, :], zv[:, :, :, 2, :],
                           yv2[:, :, :, 4, 0, :], ALU.add)
                        # l=1: n 1..3
                        TT(zv[:, :, :, 1, :], yv2[:, :, :, 0, 1, :],
                           yv2[:, :, :, 1, 0, :], ALU.add)
                        TT(zv[:, :, :, 1, :], zv[:, :, :, 1, :],
                           yv2[:, :, :, 1, 1, :], ALU.add)
                        # sqrt(x + eps): l=0 from ysq n=0; l>=1 in place
                        nc.scalar.activation(zv[:, :, :, 0, :],
                                             yv[:, :, :, 0, :], AF.Sqrt,
                                             bias=eps_t[:])
                        nc.scalar.activation(zv[:, :, :, 1:4, :],
                                             zv[:, :, :, 1:4, :], AF.Sqrt,
                                             bias=eps_t[:])

                    # --- MM3: out[pair, (par, i)] --------------------------
                    ps3 = ps3p.tile([128, 2 * COUT], F32, tag="ps3")
                    for par in range(2 if 'mm3' not in _SKIP else 0):
                        for rl in range(8):
                            lhsT = zv[64 * par:64 * par + 64, :, :, rl // 2,
                                      rl % 2]
                            rhs = w_t[64 * par:64 * par + 64,
                                      COUT * rl:COUT * (rl + 1)]
                            nc.tensor.matmul(
                                ps3[:, COUT * par:COUT * (par + 1)],
                                lhsT, rhs, start=(rl == 0), stop=(rl == 7),
                                skip_group_check=True)

                    # --- relu (+bias) + store ------------------------------
                    osb = osbp.tile([128, 2 * COUT], F32, tag="osb")
                    if with_bias:
                        for par in range(2):
                            nc.vector.tensor_add(
                                osb[:, COUT * par:COUT * (par + 1)],
                                ps3[:, COUT * par:COUT * (par + 1)],
                                bias_t[:, :].broadcast(0, 128))
                        nc.scalar.activation(osb[:], osb[:], AF.Relu)
                    else:
                        nc.gpsimd.tensor_scalar_max(osb[:], ps3[:], 0.0)
                    if 'store' not in _SKIP:
                        dst = outd.ap()[sgb + 256 * h:sgb + 256 * (h + 1), :]
                        dst = dst.rearrange("(pl par) i -> pl par i", par=2)
                        nc.sync.dma_start(
                            dst, osb[:, :].rearrange("p (par i) -> p par i",
                                                     par=2))

    nc.compile()
    return nc


# gather row i -> (v-offset-in-sg, p): part = i%128, chunk = i//128,
# pair q = 4*(chunk//2) + part//32, v = 2q + chunk%2, p = part%32
_II = np.arange(VSG * P)
_VOFF = 2 * (4 * ((_II // 128) // 2) + (_II % 128) // 32) + (_II // 128) % 2
_POFF = (_II % 128) % 32


def _prep_inputs_core(b, signal, patches_idx, conv_kernel, kernel_weights,
                      biases):
    bf = mybir.dt.np(BF16)
    sigrow = np.zeros((V, 128), dtype=bf)
    sigrow[:, 0:CIN] = signal[b].astype(bf)
    sig_u64 = np.ascontiguousarray(sigrow).view(np.uint64)   # [V, 32]

    pidx = patches_idx[b, :, :, 1]
    krn = conv_kernel[b].transpose(0, 1, 3, 2).reshape(V, P, 2 * NSH)
    # kre[sg, 32j+p, 64t + 32b + nr]: b=0: K[vA], b=1: K[vB]-K[vA],
    # with q = 4t+j, vA = sg*512 + 2q, vB = vA + 1
    ka = krn[0::2].reshape(NSG, NSLAB * 2, P, 2 * NSH)       # [sg, q, p, nr]
    kd = (krn[1::2] - krn[0::2]).reshape(NSG, NSLAB * 2, P, 2 * NSH)
    kab = np.stack([ka, kd], axis=3)          # [sg, q, p, b, nr]
    k6 = kab.reshape(NSG, 64, 4, P, 2, 2 * NSH)   # [sg, t, j, p, b, nr]
    kre_ = np.ascontiguousarray(k6.transpose(0, 2, 3, 1, 4, 5)).reshape(
        NSG, 128, NSLAB, 2 * NSH).astype(bf)

    # idx: one op per sg, 16384 idxs; arr[i] = pidx[v(i), p(i)]
    # (rows 16..127 are zero padding -- the gather ucode reads rows 0:16;
    # loading a full-height tensor avoids an uninitialized-SBUF memset)
    idxh = np.zeros((128, V * P // 16), dtype=np.int16)
    ncols = (V * P // 16) // NSG
    for sg in range(NSG):
        arr = pidx[VSG * sg + _VOFF, _POFF].astype(np.int16)
        idxh[:16, ncols * sg:ncols * (sg + 1)] = arr.reshape(ncols, 16).T

    w = kernel_weights.transpose(1, 3, 2, 0).reshape(CIN, 8 * COUT)
    wsb = np.concatenate([w, w], axis=0).astype(bf)
    bia = biases.reshape(1, COUT).astype(np.float32)
    return {"sig": sig_u64, "kre": kre_, "idx": idxh, "wsb": wsb, "bia": bia}


def _make_runner(nc, n_cores=8):
    import jax
    from jax.sharding import Mesh, PartitionSpec
    from jax.experimental.shard_map import shard_map

    bass2jax.install_neuronx_cc_hook()
    partition_name = (nc.partition_id_tensor.name
                      if nc.partition_id_tensor else None)
    in_names, out_names, out_avals, zero_outs = [], [], [], []
    for alloc in nc.m.functions[0].allocations:
        if not isinstance(alloc, mybir.MemoryLocationSet):
            continue
        name = alloc.memorylocations[0].name
        if alloc.kind == "ExternalInput":
            if name != partition_name:
                in_names.append(name)
        elif alloc.kind == "ExternalOutput":
            out_names.append(name)
            shape = tuple(alloc.tensor_shape)
            dtype = mybir.dt.np(alloc.dtype)
            out_avals.append(jax.core.ShapedArray(shape, dtype))
            zero_outs.append(np.zeros(shape, dtype))
    n_params, n_outs = len(in_names), len(out_avals)
    in_names_all = list(in_names) + list(out_names)
    if partition_name is not None:
        in_names_all.append(partition_name)

    def _body(*args):
        operands = list(args)
        if partition_name is not None:
            operands.append(bass2jax.partition_id_tensor())
        outs = bass2jax._bass_exec_p.bind(
            *operands, out_avals=tuple(out_avals),
            in_names=tuple(in_names_all), out_names=tuple(out_names),
            lowering_input_output_aliases=(),
            sim_require_finite=True, sim_require_nnan=True, nc=nc)
        return tuple(outs)

    donate = tuple(range(n_params, n_params + n_outs))
    devices = jax.devices()[:n_cores]
    mesh = Mesh(np.asarray(devices), ("core",))
    sharded = jax.jit(
        shard_map(_body, mesh=mesh,
                  in_specs=(PartitionSpec("core"),) * (n_params + n_outs),
                  out_specs=(PartitionSpec("core"),) * n_outs,
                  check_rep=False),
        donate_argnums=donate, keep_unused=True)

    def run_fn(in_maps):
        import jax
        per_core = [[np.asarray(m[nm]) for nm in in_names] for m in in_maps]
        concat_in = [
            np.concatenate([per_core[c][i] for c in range(n_cores)], axis=0)
            for i in range(n_params)]
        concat_zeros = [
            np.zeros((n_cores * z.shape[0], *z.shape[1:]), z.dtype)
            for z in zero_outs]
        out_arrs = sharded(*concat_in, *concat_zeros)
        jax.block_until_ready(out_arrs)
        return [
            {nm: np.asarray(out_arrs[i]).reshape(n_cores, *out_avals[i].shape)[c]
             for i, nm in enumerate(out_names)}
            for c in range(n_cores)]

    return run_fn


def kernel(signal, patches_idx, conv_kernel, kernel_weights, biases):
    with_bias = bool(np.any(biases))
    key = ("k", with_bias)
    if key not in _CACHE:
        nc = _build_nc(with_bias)
        _CACHE[key] = (nc, _make_runner(nc))
    nc, run = _CACHE[key]

    in_maps = []
    for b in range(B):
        m = _prep_inputs_core(b, signal, patches_idx, conv_kernel,
                              kernel_weights, biases)
        in_maps.append(m)

    results = run(in_maps)
    out = np.stack([results[b]["outd"] for b in range(B)], axis=0)
    return out.astype(np.float32)


# revision 56
# speedup vs baseline: 1.0306x; 1.0012x over previous
"""Trainium2 Bass kernel for nn_BinaryTreeShInvariantConv.

Per (b, v): gather P=32 neighbor rows of signal[b] (Cin=64), contract over P
against conv_kernel[b,v] -> y[Cin, R*N], square, sum SH orders per degree l,
sqrt(+eps), contract [Cin*R*(L+1)=512] against kernel_weights -> [Cout=128],
bias + relu.

Sharding: data-parallel over batch B=8 -> one batch per NeuronCore (SPMD).

Design (driven by the CoreSim v1 cost model, which prices each instruction
as free-size x engine-cycle charged serially to its issuing engine):
  - Gather reads bf16 rows PACKED AS uint64 (16 u64 = 64 bf16 channels):
    the gather is priced as a generic Pool op at out-free-ELEMENTS x 0.83ns,
    so 8x fewer elements -> 4 ops x 427ns per 512-v supergroup (13.7us total
    vs 218us naive).
  - "Pair-diagonal" lhsT: gathered rows land directly in block-diagonal
    [64 part, 128 col] bf16 slabs (2 v's per slab; off-diag zeros memset
    once per buffer). MM1 -> [128 part = (v-parity, c), 32 rn] per pair:
    half the PE columns of a 4-v block-diag rhs; conv_kernel needs one
    [128, 4096] DMA per supergroup.
  - Degree sums as strided bf16 tensor_tensor adds (2x DVE mode) instead of
    reduce_sum (no fast mode).
  - Squares (PSUM f32 -> bf16) split between ACT (activation Square) and
    Pool (tensor_tensor mult) to balance engine occupancy.
  - MM3 packs 128 v-pairs in output partitions.

Supergroup = 512 v's (NSG=8): 4 gather ops (one per lhsT quadrant class,
4096 idxs each), 1 kre DMA, 8x32 MM1 matmuls -> 8 PSUM granules -> squares
-> per 256-v block: DVE degree-adds, ACT sqrt(+eps), MM3 (2 par x 8 rl
accumulating), DVE relu, store.
"""

import sys

sys.path.insert(0, "/opt/trn_rl_repo")

import numpy as np

import concourse.bacc as bacc
import concourse.mybir as mybir
import concourse.tile as tile
from concourse import ap_utils
from concourse.bass import MemorySpace
from concourse import bass2jax

B, V, P, CIN, R, COUT = 8, 4096, 32, 64, 2, 128
NSH, NDEG = 16, 4
VSG = 512            # v's per supergroup
NSG = V // VSG       # 8 supergroups
NSLAB = 128          # v-pairs per part-block (h) per supergroup
# engine per square granule (gidx = 4h+gl): balance ACT/Pool/DVE
SQ_ENG = [list("PADPAPAP"), list("PADPAPAP")]
BF16 = mybir.dt.bfloat16
F32 = mybir.dt.float32
I16 = mybir.dt.int16
U64 = mybir.dt.uint64

_CACHE = {}
_SKIP = set()  # debug: subset of {'gather','mm1','post','mm3','store'}


def _dma_gather_any(eng, out_ap, in_ap, idxs_ap, num_idxs, elem_size,
                    single_packet=True, nreg=None):
    """bass.dma_gather with relaxed asserts. The executor flattens the out AP
    and reshapes it to (128, ceil(n/128), elem) as a VIEW: row i lands at
    flat slot (i%128)*chunks + i//128. The AP's stride tree must therefore be
    numpy-reshape-compatible with that shape (seamless merges)."""
    assert idxs_ap.dtype == I16
    assert in_ap.space == MemorySpace.DRAM
    assert in_ap.dtype == out_ap.dtype
    elem_step = in_ap.ap[0][0]
    stride_bytes = elem_step * mybir.dt.size(in_ap.dtype)
    assert stride_bytes % 256 == 0 and stride_bytes // 256 < 256
    assert ap_utils.ap_is_contiguous(idxs_ap.ap[1:])
    assert in_ap.ap[-1][1] == elem_size
    total = 1
    for _, c in out_ap.ap:
        total *= c
    assert total == ((num_idxs + 127) // 128) * 128 * elem_size

    _in_ap = eng.lower_ap_dma(in_ap, for_custom_bir_dma=True)
    if nreg is None:
        nreg = eng.to_reg(num_idxs)
    return eng.add_instruction(
        mybir.InstDMAGatherAnt(
            name=eng.bass.get_next_instruction_name(),
            ins=[*_in_ap, eng.lower_ap(idxs_ap),
                 eng.lower_val_access(nreg)],
            outs=[eng.lower_ap(out_ap)],
            transpose=False,
            num_idxs=num_idxs,
            elem_size=elem_size,
            stride_bytes_256=stride_bytes // 256,
            gen_mode=0,
            single_packet=single_packet,
            queue_num=0,
            sbuf_tokens_per_rank=0,
            sbuf_free_dim_per_rank=0,
            sbuf_free_dim_pad_per_rank=0,
            sbuf_byte_offset=0,
        ))


def _build_nc(with_bias):
    nc = bacc.Bacc("TRN2", target_bir_lowering=False, debug=False,
                   enable_asserts=False, dynamic_dma_scratch_size=49152)
    AF = mybir.ActivationFunctionType
    ALU = mybir.AluOpType

    sig = nc.dram_tensor("sig", [V, 64], mybir.dt.uint32,
                         kind="ExternalInput")
    kre = nc.dram_tensor("kre", [NSG, 128, NSLAB, 2 * NSH], BF16,
                         kind="ExternalInput")
    idx = nc.dram_tensor("idx", [128, V * P // 16], I16, kind="ExternalInput")
    wsb = nc.dram_tensor("wsb", [128, 8 * COUT], BF16, kind="ExternalInput")
    bia = nc.dram_tensor("bia", [1, COUT], F32, kind="ExternalInput")
    outd = nc.dram_tensor("outd", [V, COUT], F32, kind="ExternalOutput")

    with tile.TileContext(nc) as tc:
        with (
            tc.tile_pool(name="const", bufs=1) as constp,
            tc.tile_pool(name="patbf", bufs=2) as patbfp,
            tc.tile_pool(name="kre", bufs=3) as krep,
            tc.tile_pool(name="ysq", bufs=4) as ysqp,
            tc.tile_pool(name="zsb", bufs=2) as zsbp,
            tc.tile_pool(name="zt", bufs=2) as ztp,
            tc.tile_pool(name="osb", bufs=2) as osbp,
            tc.tile_pool(name="ps1", bufs=6, space="PSUM") as ps1p,
            tc.tile_pool(name="ps3", bufs=2, space="PSUM") as ps3p,
        ):
            w_t = constp.tile([128, 8 * COUT], BF16, tag="w")
            idx_t = constp.tile([128, V * P // 16], I16, tag="idx")
            ISG = (V * P // 16) // NSG
            if with_bias:
                bias_t = constp.tile([1, COUT], F32, tag="bias")
                nc.sync.dma_start(bias_t[:], bia.ap())
            eps_t = constp.tile([128, 1], F32, tag="eps")
            nc.vector.memset(eps_t[:], 1e-4)

            # dense patch tiles: fully rewritten by the gather each
            # supergroup, so no zeroing needed. Row (part, chunk) holds the
            # patch row of (v, p) = (sgb + 2*(4*(chunk//2) + part//32)
            # + chunk%2, part%32).
            patbfs = [patbfp.tile([128, NSLAB * 64], BF16, tag="patbf",
                                  name=f"patbf_{i}") for i in range(2)]

            nreg = nc.gpsimd.to_reg(VSG * P)
            nreg0 = nc.gpsimd.to_reg(2048)
            nreg1 = nc.gpsimd.to_reg(VSG * P - 2048)
            sig64 = sig.ap().bitcast(U64)

            for sg in range(NSG):
                sgb = VSG * sg
                # --- idx slice + gather: one dense 128-part op -------------
                col0 = ISG * sg
                patbf = patbfs[sg % 2]
                gv = patbf[:, :].bitcast(U64).rearrange(
                    "p (ch c) -> p ch c", ch=NSLAB)
                kre_t = krep.tile([128, NSLAB * 2 * NSH], BF16, tag="kre")
                if sg == 0:
                    # startup fast path: stage the first MM1 granule's inputs
                    # (idx cols 0:128 -> gather rows 0:2048 -> kre t 0:15)
                    # before the bulk loads so PE starts ~3us earlier.
                    nc.scalar.dma_start(idx_t[:, 0:128], idx.ap()[:, 0:128])
                    nc.scalar.dma_start(idx_t[:, 128:ISG],
                                        idx.ap()[:, 128:ISG])
                    if 'gather' not in _SKIP:
                        _dma_gather_any(
                            nc.gpsimd, gv[:, 0:16, :], sig64[:, 0:CIN // 4],
                            idx_t[:, 0:128], 2048, CIN // 4,
                            single_packet=False, nreg=nreg0)
                        _dma_gather_any(
                            nc.gpsimd, gv[:, 16:, :], sig64[:, 0:CIN // 4],
                            idx_t[:, 128:ISG], VSG * P - 2048, CIN // 4,
                            single_packet=False, nreg=nreg1)
                    nc.sync.dma_start(kre_t[:, 0:512], kre.ap()[0, :, 0:16])
                    nc.sync.dma_start(kre_t[:, 512:],
                                      kre.ap()[0, :, 16:NSLAB])
                    # w needed only by MM3; load after the critical kre0
                    nc.sync.dma_start(w_t[:], wsb.ap())
                else:
                    nc.scalar.dma_start(idx_t[:, col0:col0 + ISG],
                                        idx.ap()[:, col0:col0 + ISG])
                    if 'gather' not in _SKIP:
                        _dma_gather_any(
                            nc.gpsimd, gv, sig64[:, 0:CIN // 4],
                            idx_t[:, col0:col0 + ISG], VSG * P,
                            CIN // 4, single_packet=False, nreg=nreg)
                    nc.sync.dma_start(kre_t[:], kre.ap()[sg])

                # --- MM1 for both part-blocks first (keeps PE dense), then
                # the post chains (adds/sqrt/MM3) so MM3(h0) never head-of-
                # line blocks MM1(h1) on the in-order PE queue.
                ysqs = []
                for h in range(2):
                    ysq = ysqp.tile([128, 4 * 32 * 32], BF16, tag="ysq")
                    ysqs.append(ysq)
                    for gl in range(8):
                        ps1 = ps1p.tile([128, 512], F32, tag="ps1")
                        for ql in range(16 if 'mm1' not in _SKIP else 0):
                            q = 128 * h + 16 * gl + ql
                            j, t = q % 4, q // 4
                            # y_A||garbage: pat(vA|vB) x K_A -> [128, 32]
                            nc.tensor.matmul(
                                ps1[:, 32 * ql:32 * ql + 32],
                                patbf[32 * j:32 * j + 32,
                                      128 * t:128 * t + 128],
                                kre_t[32 * j:32 * j + 32,
                                      64 * t:64 * t + 32],
                                start=True, stop=False,
                                skip_group_check=True,
                                tile_position=(32 * j, 0))
                            # fix upper half: += pat(vB) x (K_B - K_A)
                            nc.tensor.matmul(
                                ps1[64:128, 32 * ql:32 * ql + 32],
                                patbf[32 * j:32 * j + 32,
                                      128 * t + 64:128 * t + 128],
                                kre_t[32 * j:32 * j + 32,
                                      64 * t + 32:64 * t + 64],
                                start=False, stop=True,
                                skip_group_check=True,
                                tile_position=(32 * j, 64))
                        if 'post' not in _SKIP:
                            dst = ysq[:, 512 * gl:512 * (gl + 1)]
                            se = SQ_ENG[sg % 2][(8 * h + gl) % 8]
                            if se == 'A':
                                nc.scalar.activation(dst, ps1[:], AF.Square)
                            else:
                                e = nc.gpsimd if se == 'P' else nc.vector
                                e.tensor_tensor(dst, ps1[:], ps1[:], ALU.mult)

                for h in range(2):
                    ysq = ysqs[h]
                    # ysq: [128, (g 4, pl 32, n 16, r 2)]
                    yv = ysq[:, :].rearrange("p (g pl n r) -> p g pl n r",
                                             g=4, pl=32, n=NSH)
                    # n split (n2, two): n = 2*n2 + two, for stride-2 windows
                    yv2 = ysq[:, :].rearrange(
                        "p (g pl n2 two r) -> p g pl n2 two r",
                        g=4, pl=32, n2=NSH // 2, two=2)
                    # zsb: [128, (g 4, pl 32, l 4, r 2)]
                    zsb = zsbp.tile([128, 4 * 32 * NDEG * 2], BF16, tag="zsb")
                    zv = zsb[:, :].rearrange("p (g pl l r) -> p g pl l r",
                                             g=4, pl=32, l=NDEG)
                    # zt scratch: [128, (g 4, pl 32, t 5, r 2)]
                    zt = ztp.tile([128, 4 * 32 * 5 * 2], BF16, tag="zt")
                    tv = zt[:, :].rearrange("p (g pl t r) -> p g pl t r",
                                            g=4, pl=32, t=5)
                    # last block's adds on Pool so the final MM3 doesn't
                    # wait behind DVE's queue at drain time
                    TT = (nc.gpsimd.tensor_tensor
                          if (sg == NSG - 1 and h == 1)
                          else nc.vector.tensor_tensor)
                    if 'post' not in _SKIP:
                        # l=3: n 9..15 (7 terms): {9,11,13}+{10,12,14}, +15
                        TT(tv[:, :, :, 0:3, :], yv2[:, :, :, 4:7, 1, :],
                           yv2[:, :, :, 5:8, 0, :], ALU.add)
                        TT(zv[:, :, :, 3, :], tv[:, :, :, 0, :],
                           tv[:, :, :, 1, :], ALU.add)
                        TT(zv[:, :, :, 3, :], zv[:, :, :, 3, :],
                           tv[:, :, :, 2, :], ALU.add)
                        TT(zv[:, :, :, 3, :], zv[:, :, :, 3, :],
                           yv2[:, :, :, 7, 1, :], ALU.add)
                        # l=2: n 4..8 (5 terms): {4,6}+{5,7}, +8
                        TT(tv[:, :, :, 3:5, :], yv2[:, :, :, 2:4, 0, :],
                           yv2[:, :, :, 2:4, 1, :], ALU.add)
                        TT(zv[:, :, :, 2, :], tv[:, :, :, 3, :],
                           tv[:, :, :, 4, :], ALU.add)
                        TT(zv[:, :, :, 2, :], zv[:, :, :, 2, :],
                           yv2[:, :, :, 4, 0, :], ALU.add)
                        # l=1: n 1..3
                        TT(zv[:, :, :, 1, :], yv2[:, :, :, 0, 1, :],
                           yv2[:, :, :, 1, 0, :], ALU.add)
                        TT(zv[:, :, :, 1, :], zv[:, :, :, 1, :],
                           yv2[:, :, :, 1, 1, :], ALU.add)
                        # sqrt(x + eps): l=0 from ysq n=0; l>=1 in place
                        nc.scalar.activation(zv[:, :, :, 0, :],
                                             yv[:, :, :, 0, :], AF.Sqrt,
                                             bias=eps_t[:])
                        nc.scalar.activation(zv[:, :, :, 1:4, :],
                                             zv[:, :, :, 1:4, :], AF.Sqrt,
                                             bias=eps_t[:])

                    # --- MM3: out[pair, (par, i)] --------------------------
                    ps3 = ps3p.tile([128, 2 * COUT], F32, tag="ps3")
                    for par in range(2 if 'mm3' not in _SKIP else 0):
                        for rl in range(8):
                            lhsT = zv[64 * par:64 * par + 64, :, :, rl // 2,
                                      rl % 2]
                            rhs = w_t[64 * par:64 * par + 64,
                                      COUT * rl:COUT * (rl + 1)]
                            nc.tensor.matmul(
                                ps3[:, COUT * par:COUT * (par + 1)],
                                lhsT, rhs, start=(rl == 0), stop=(rl == 7),
                                skip_group_check=True)

                    # --- relu (+bias) + store ------------------------------
                    osb = osbp.tile([128, 2 * COUT], F32, tag="osb")
                    if with_bias:
                        for par in range(2):
                            nc.vector.tensor_add(
                                osb[:, COUT * par:COUT * (par + 1)],
                                ps3[:, COUT * par:COUT * (par + 1)],
                                bias_t[:, :].broadcast(0, 128))
                        nc.scalar.activation(osb[:], osb[:], AF.Relu)
                    else:
                        nc.gpsimd.tensor_scalar_max(osb[:], ps3[:], 0.0)
                    if 'store' not in _SKIP:
                        dst = outd.ap()[sgb + 256 * h:sgb + 256 * (h + 1), :]
                        dst = dst.rearrange("(pl par) i -> pl par i", par=2)
                        nc.sync.dma_start(
                            dst, osb[:, :].rearrange("p (par i) -> p par i",
                                                     par=2))

    nc.compile()
    return nc


# gather row i -> (v-offset-in-sg, p): part = i%128, chunk = i//128,
# pair q = 4*(chunk//2) + part//32, v = 2q + chunk%2, p = part%32
_II = np.arange(VSG * P)
_VOFF = 2 * (4 * ((_II // 128) // 2) + (_II % 128) // 32) + (_II // 128) % 2
_POFF = (_II % 128) % 32


def _prep_inputs_core(b, signal, patches_idx, conv_kernel, kernel_weights,
                      biases):
    bf = mybir.dt.np(BF16)
    sigrow = np.zeros((V, 128), dtype=bf)
    sigrow[:, 0:CIN] = signal[b].astype(bf)
    sig_u32 = np.ascontiguousarray(sigrow).view(np.uint32)   # [V, 64]

    pidx = patches_idx[b, :, :, 1]
    krn = conv_kernel[b].transpose(0, 1, 3, 2).reshape(V, P, 2 * NSH)
    # kre[sg, 32j+p, 64t + 32b + nr]: b=0: K[vA], b=1: K[vB]-K[vA],
    # with q = 4t+j, vA = sg*512 + 2q, vB = vA + 1
    ka = krn[0::2].reshape(NSG, NSLAB * 2, P, 2 * NSH)       # [sg, q, p, nr]
    kd = (krn[1::2] - krn[0::2]).reshape(NSG, NSLAB * 2, P, 2 * NSH)
    kab = np.stack([ka, kd], axis=3)          # [sg, q, p, b, nr]
    k6 = kab.reshape(NSG, 64, 4, P, 2, 2 * NSH)   # [sg, t, j, p, b, nr]
    kre_ = np.ascontiguousarray(k6.transpose(0, 2, 3, 1, 4, 5)).reshape(
        NSG, 128, NSLAB, 2 * NSH).astype(bf)

    # idx: one op per sg, 16384 idxs; arr[i] = pidx[v(i), p(i)]
    # (rows 16..127 are zero padding -- the gather ucode reads rows 0:16;
    # loading a full-height tensor avoids an uninitialized-SBUF memset)
    idxh = np.zeros((128, V * P // 16), dtype=np.int16)
    ncols = (V * P // 16) // NSG
    for sg in range(NSG):
        arr = pidx[VSG * sg + _VOFF, _POFF].astype(np.int16)
        idxh[:16, ncols * sg:ncols * (sg + 1)] = arr.reshape(ncols, 16).T

    w = kernel_weights.transpose(1, 3, 2, 0).reshape(CIN, 8 * COUT)
    wsb = np.concatenate([w, w], axis=0).astype(bf)
    bia = biases.reshape(1, COUT).astype(np.float32)
    return {"sig": sig_u32, "kre": kre_, "idx": idxh, "wsb": wsb, "bia": bia}


def _make_runner(nc, n_cores=8):
    import jax
    from jax.sharding import Mesh, PartitionSpec
    from jax.experimental.shard_map import shard_map

    bass2jax.install_neuronx_cc_hook()
    partition_name = (nc.partition_id_tensor.name
                      if nc.partition_id_tensor else None)
    in_names, out_names, out_avals, zero_outs = [], [], [], []
    for alloc in nc.m.functions[0].allocations:
        if not isinstance(alloc, mybir.MemoryLocationSet):
            continue
        name = alloc.memorylocations[0].name
        if alloc.kind == "ExternalInput":
            if name != partition_name:
                in_names.append(name)
        elif alloc.kind == "ExternalOutput":
            out_names.append(name)
            shape = tuple(alloc.tensor_shape)
            dtype = mybir.dt.np(alloc.dtype)
            out_avals.append(jax.core.ShapedArray(shape, dtype))
            zero_outs.append(np.zeros(shape, dtype))
    n_params, n_outs = len(in_names), len(out_avals)
    in_names_all = list(in_names) + list(out_names)
    if partition_name is not None:
        in_names_all.append(partition_name)

    def _body(*args):
        operands = list(args)
        if partition_name is not None:
            operands.append(bass2jax.partition_id_tensor())
        outs = bass2jax._bass_exec_p.bind(
            *operands, out_avals=tuple(out_avals),
            in_names=tuple(in_names_all), out_names=tuple(out_names),
            lowering_input_output_aliases=(),
            sim_require_finite=True, sim_require_nnan=True, nc=nc)
        return tuple(outs)

    donate = tuple(range(n_params, n_params + n_outs))
    devices = jax.devices()[:n_cores]
    mesh = Mesh(np.asarray(devices), ("core",))
    sharded = jax.jit(
        shard_map(_body, mesh=mesh,
                  in_specs=(PartitionSpec("core"),) * (n_params + n_outs),
                  out_specs=(PartitionSpec("core"),) * n_outs,
                  check_rep=False),
        donate_argnums=donate, keep_unused=True)

    def run_fn(in_maps):
        import jax
        per_core = [[np.asarray(m[nm]) for nm in in_names] for m in in_maps]
        concat_in = [
            np.concatenate([per_core[c][i] for c in range(n_cores)], axis=0)
            for i in range(n_params)]
        concat_zeros = [
            np.zeros((n_cores * z.shape[0], *z.shape[1:]), z.dtype)
            for z in zero_outs]
        out_arrs = sharded(*concat_in, *concat_zeros)
        jax.block_until_ready(out_arrs)
        return [
            {nm: np.asarray(out_arrs[i]).reshape(n_cores, *out_avals[i].shape)[c]
             for i, nm in enumerate(out_names)}
            for c in range(n_cores)]

    return run_fn


def kernel(signal, patches_idx, conv_kernel, kernel_weights, biases):
    with_bias = bool(np.any(biases))
    key = ("k", with_bias)
    if key not in _CACHE:
        nc = _build_nc(with_bias)
        _CACHE[key] = (nc, _make_runner(nc))
    nc, run = _CACHE[key]

    in_maps = []
    for b in range(B):
        m = _prep_inputs_core(b, signal, patches_idx, conv_kernel,
                              kernel_weights, biases)
        in_maps.append(m)

    results = run(in_maps)
    out = np.stack([results[b]["outd"] for b in range(B)], axis=0)
    return out.astype(np.float32)


# revision 57
# speedup vs baseline: 1.0364x; 1.0056x over previous
"""Trainium2 Bass kernel for nn_BinaryTreeShInvariantConv.

Per (b, v): gather P=32 neighbor rows of signal[b] (Cin=64), contract over P
against conv_kernel[b,v] -> y[Cin, R*N], square, sum SH orders per degree l,
sqrt(+eps), contract [Cin*R*(L+1)=512] against kernel_weights -> [Cout=128],
bias + relu.

Sharding: data-parallel over batch B=8 -> one batch per NeuronCore (SPMD).

Design (driven by the CoreSim v1 cost model, which prices each instruction
as free-size x engine-cycle charged serially to its issuing engine):
  - Gather reads bf16 rows PACKED AS uint64 (16 u64 = 64 bf16 channels):
    the gather is priced as a generic Pool op at out-free-ELEMENTS x 0.83ns,
    so 8x fewer elements -> 4 ops x 427ns per 512-v supergroup (13.7us total
    vs 218us naive).
  - "Pair-diagonal" lhsT: gathered rows land directly in block-diagonal
    [64 part, 128 col] bf16 slabs (2 v's per slab; off-diag zeros memset
    once per buffer). MM1 -> [128 part = (v-parity, c), 32 rn] per pair:
    half the PE columns of a 4-v block-diag rhs; conv_kernel needs one
    [128, 4096] DMA per supergroup.
  - Degree sums as strided bf16 tensor_tensor adds (2x DVE mode) instead of
    reduce_sum (no fast mode).
  - Squares (PSUM f32 -> bf16) split between ACT (activation Square) and
    Pool (tensor_tensor mult) to balance engine occupancy.
  - MM3 packs 128 v-pairs in output partitions.

Supergroup = 512 v's (NSG=8): 4 gather ops (one per lhsT quadrant class,
4096 idxs each), 1 kre DMA, 8x32 MM1 matmuls -> 8 PSUM granules -> squares
-> per 256-v block: DVE degree-adds, ACT sqrt(+eps), MM3 (2 par x 8 rl
accumulating), DVE relu, store.
"""

import sys

sys.path.insert(0, "/opt/trn_rl_repo")

import numpy as np

import concourse.bacc as bacc
import concourse.mybir as mybir
import concourse.tile as tile
from concourse import ap_utils
from concourse.bass import MemorySpace
from concourse import bass2jax

B, V, P, CIN, R, COUT = 8, 4096, 32, 64, 2, 128
NSH, NDEG = 16, 4
VSG = 512            # v's per supergroup
NSG = V // VSG       # 8 supergroups
NSLAB = 128          # v-pairs per part-block (h) per supergroup
# engine per square granule (gidx = 4h+gl): balance ACT/Pool/DVE
SQ_ENG = [list("DADADADA"), list("DADADAAD")]
BF16 = mybir.dt.bfloat16
F32 = mybir.dt.float32
I16 = mybir.dt.int16
U64 = mybir.dt.uint64

_CACHE = {}
_SKIP = set()  # debug: subset of {'gather','mm1','post','mm3','store'}


def _dma_gather_any(eng, out_ap, in_ap, idxs_ap, num_idxs, elem_size,
                    single_packet=True, nreg=None):
    """bass.dma_gather with relaxed asserts. The executor flattens the out AP
    and reshapes it to (128, ceil(n/128), elem) as a VIEW: row i lands at
    flat slot (i%128)*chunks + i//128. The AP's stride tree must therefore be
    numpy-reshape-compatible with that shape (seamless merges)."""
    assert idxs_ap.dtype == I16
    assert in_ap.space == MemorySpace.DRAM
    assert in_ap.dtype == out_ap.dtype
    elem_step = in_ap.ap[0][0]
    stride_bytes = elem_step * mybir.dt.size(in_ap.dtype)
    assert stride_bytes % 256 == 0 and stride_bytes // 256 < 256
    assert ap_utils.ap_is_contiguous(idxs_ap.ap[1:])
    assert in_ap.ap[-1][1] == elem_size
    total = 1
    for _, c in out_ap.ap:
        total *= c
    assert total == ((num_idxs + 127) // 128) * 128 * elem_size

    _in_ap = eng.lower_ap_dma(in_ap, for_custom_bir_dma=True)
    if nreg is None:
        nreg = eng.to_reg(num_idxs)
    return eng.add_instruction(
        mybir.InstDMAGatherAnt(
            name=eng.bass.get_next_instruction_name(),
            ins=[*_in_ap, eng.lower_ap(idxs_ap),
                 eng.lower_val_access(nreg)],
            outs=[eng.lower_ap(out_ap)],
            transpose=False,
            num_idxs=num_idxs,
            elem_size=elem_size,
            stride_bytes_256=stride_bytes // 256,
            gen_mode=0,
            single_packet=single_packet,
            queue_num=0,
            sbuf_tokens_per_rank=0,
            sbuf_free_dim_per_rank=0,
            sbuf_free_dim_pad_per_rank=0,
            sbuf_byte_offset=0,
        ))


def _build_nc(with_bias):
    nc = bacc.Bacc("TRN2", target_bir_lowering=False, debug=False,
                   enable_asserts=False, dynamic_dma_scratch_size=49152)
    AF = mybir.ActivationFunctionType
    ALU = mybir.AluOpType

    sig = nc.dram_tensor("sig", [V, 64], mybir.dt.uint32,
                         kind="ExternalInput")
    kre = nc.dram_tensor("kre", [NSG, 128, NSLAB, 2 * NSH], BF16,
                         kind="ExternalInput")
    idx = nc.dram_tensor("idx", [128, V * P // 16], I16, kind="ExternalInput")
    wsb = nc.dram_tensor("wsb", [128, 8 * COUT], BF16, kind="ExternalInput")
    bia = nc.dram_tensor("bia", [1, COUT], F32, kind="ExternalInput")
    outd = nc.dram_tensor("outd", [V, COUT], F32, kind="ExternalOutput")

    with tile.TileContext(nc) as tc:
        with (
            tc.tile_pool(name="const", bufs=1) as constp,
            tc.tile_pool(name="patbf", bufs=2) as patbfp,
            tc.tile_pool(name="kre", bufs=3) as krep,
            tc.tile_pool(name="ysq", bufs=4) as ysqp,
            tc.tile_pool(name="zsb", bufs=2) as zsbp,
            tc.tile_pool(name="zt", bufs=2) as ztp,
            tc.tile_pool(name="osb", bufs=2) as osbp,
            tc.tile_pool(name="ps1", bufs=6, space="PSUM") as ps1p,
            tc.tile_pool(name="ps3", bufs=2, space="PSUM") as ps3p,
        ):
            w_t = constp.tile([128, 8 * COUT], BF16, tag="w")
            idx_t = constp.tile([128, V * P // 16], I16, tag="idx")
            ISG = (V * P // 16) // NSG
            if with_bias:
                bias_t = constp.tile([1, COUT], F32, tag="bias")
                nc.sync.dma_start(bias_t[:], bia.ap())
            eps_t = constp.tile([128, 1], F32, tag="eps")
            nc.vector.memset(eps_t[:], 1e-4)

            # dense patch tiles: fully rewritten by the gather each
            # supergroup, so no zeroing needed. Row (part, chunk) holds the
            # patch row of (v, p) = (sgb + 2*(4*(chunk//2) + part//32)
            # + chunk%2, part%32).
            patbfs = [patbfp.tile([128, NSLAB * 64], BF16, tag="patbf",
                                  name=f"patbf_{i}") for i in range(2)]

            nreg = nc.gpsimd.to_reg(VSG * P)
            nreg0 = nc.gpsimd.to_reg(2048)
            nreg1 = nc.gpsimd.to_reg(VSG * P - 2048)
            sig64 = sig.ap().bitcast(U64)

            for sg in range(NSG):
                sgb = VSG * sg
                # --- idx slice + gather: one dense 128-part op -------------
                col0 = ISG * sg
                patbf = patbfs[sg % 2]
                gv = patbf[:, :].bitcast(U64).rearrange(
                    "p (ch c) -> p ch c", ch=NSLAB)
                kre_t = krep.tile([128, NSLAB * 2 * NSH], BF16, tag="kre")
                if sg == 0:
                    # startup fast path: stage the first MM1 granule's inputs
                    # (idx cols 0:128 -> gather rows 0:2048 -> kre t 0:15)
                    # before the bulk loads so PE starts ~3us earlier.
                    nc.sync.dma_start(idx_t[:, 0:128], idx.ap()[:, 0:128])
                    nc.scalar.dma_start(idx_t[:, 128:ISG],
                                        idx.ap()[:, 128:ISG])
                    if 'gather' not in _SKIP:
                        _dma_gather_any(
                            nc.gpsimd, gv[:, 0:16, :], sig64[:, 0:CIN // 4],
                            idx_t[:, 0:128], 2048, CIN // 4,
                            single_packet=False, nreg=nreg0)
                        _dma_gather_any(
                            nc.gpsimd, gv[:, 16:, :], sig64[:, 0:CIN // 4],
                            idx_t[:, 128:ISG], VSG * P - 2048, CIN // 4,
                            single_packet=False, nreg=nreg1)
                    nc.sync.dma_start(kre_t[:, 0:512], kre.ap()[0, :, 0:16])
                    nc.sync.dma_start(kre_t[:, 512:],
                                      kre.ap()[0, :, 16:NSLAB])
                    # w needed only by MM3; load after the critical kre0
                    nc.sync.dma_start(w_t[:], wsb.ap())
                else:
                    nc.scalar.dma_start(idx_t[:, col0:col0 + ISG],
                                        idx.ap()[:, col0:col0 + ISG])
                    if 'gather' not in _SKIP:
                        _dma_gather_any(
                            nc.gpsimd, gv, sig64[:, 0:CIN // 4],
                            idx_t[:, col0:col0 + ISG], VSG * P,
                            CIN // 4, single_packet=False, nreg=nreg)
                    nc.sync.dma_start(kre_t[:], kre.ap()[sg])

                # --- MM1 for both part-blocks first (keeps PE dense), then
                # the post chains (adds/sqrt/MM3) so MM3(h0) never head-of-
                # line blocks MM1(h1) on the in-order PE queue.
                ysqs = []
                for h in range(2):
                    ysq = ysqp.tile([128, 4 * 32 * 32], BF16, tag="ysq")
                    ysqs.append(ysq)
                    for gl in range(8):
                        ps1 = ps1p.tile([128, 512], F32, tag="ps1")
                        for ql in range(16 if 'mm1' not in _SKIP else 0):
                            q = 128 * h + 16 * gl + ql
                            j, t = q % 4, q // 4
                            # y_A||garbage: pat(vA|vB) x K_A -> [128, 32]
                            nc.tensor.matmul(
                                ps1[:, 32 * ql:32 * ql + 32],
                                patbf[32 * j:32 * j + 32,
                                      128 * t:128 * t + 128],
                                kre_t[32 * j:32 * j + 32,
                                      64 * t:64 * t + 32],
                                start=True, stop=False,
                                skip_group_check=True,
                                tile_position=(32 * j, 0))
                            # fix upper half: += pat(vB) x (K_B - K_A)
                            nc.tensor.matmul(
                                ps1[64:128, 32 * ql:32 * ql + 32],
                                patbf[32 * j:32 * j + 32,
                                      128 * t + 64:128 * t + 128],
                                kre_t[32 * j:32 * j + 32,
                                      64 * t + 32:64 * t + 64],
                                start=False, stop=True,
                                skip_group_check=True,
                                tile_position=(32 * j, 64))
                        if 'post' not in _SKIP:
                            dst = ysq[:, 512 * gl:512 * (gl + 1)]
                            se = SQ_ENG[sg % 2][(8 * h + gl) % 8]
                            if se == 'A':
                                nc.scalar.activation(dst, ps1[:], AF.Square)
                            else:
                                e = nc.gpsimd if se == 'P' else nc.vector
                                e.tensor_tensor(dst, ps1[:], ps1[:], ALU.mult)

                for h in range(2):
                    ysq = ysqs[h]
                    # ysq: [128, (g 4, pl 32, n 16, r 2)]
                    yv = ysq[:, :].rearrange("p (g pl n r) -> p g pl n r",
                                             g=4, pl=32, n=NSH)
                    # n split (n2, two): n = 2*n2 + two, for stride-2 windows
                    yv2 = ysq[:, :].rearrange(
                        "p (g pl n2 two r) -> p g pl n2 two r",
                        g=4, pl=32, n2=NSH // 2, two=2)
                    # zsb: [128, (g 4, pl 32, l 4, r 2)]
                    zsb = zsbp.tile([128, 4 * 32 * NDEG * 2], BF16, tag="zsb")
                    zv = zsb[:, :].rearrange("p (g pl l r) -> p g pl l r",
                                             g=4, pl=32, l=NDEG)
                    # zt scratch: [128, (g 4, pl 32, t 5, r 2)]
                    zt = ztp.tile([128, 4 * 32 * 5 * 2], BF16, tag="zt")
                    tv = zt[:, :].rearrange("p (g pl t r) -> p g pl t r",
                                            g=4, pl=32, t=5)
                    TT = nc.gpsimd.tensor_tensor
                    if 'post' not in _SKIP:
                        # l=3: n 9..15 (7 terms): {9,11,13}+{10,12,14}, +15
                        TT(tv[:, :, :, 0:3, :], yv2[:, :, :, 4:7, 1, :],
                           yv2[:, :, :, 5:8, 0, :], ALU.add)
                        TT(zv[:, :, :, 3, :], tv[:, :, :, 0, :],
                           tv[:, :, :, 1, :], ALU.add)
                        TT(zv[:, :, :, 3, :], zv[:, :, :, 3, :],
                           tv[:, :, :, 2, :], ALU.add)
                        TT(zv[:, :, :, 3, :], zv[:, :, :, 3, :],
                           yv2[:, :, :, 7, 1, :], ALU.add)
                        # l=2: n 4..8 (5 terms): {4,6}+{5,7}, +8
                        TT(tv[:, :, :, 3:5, :], yv2[:, :, :, 2:4, 0, :],
                           yv2[:, :, :, 2:4, 1, :], ALU.add)
                        TT(zv[:, :, :, 2, :], tv[:, :, :, 3, :],
                           tv[:, :, :, 4, :], ALU.add)
                        TT(zv[:, :, :, 2, :], zv[:, :, :, 2, :],
                           yv2[:, :, :, 4, 0, :], ALU.add)
                        # l=1: n 1..3
                        TT(zv[:, :, :, 1, :], yv2[:, :, :, 0, 1, :],
                           yv2[:, :, :, 1, 0, :], ALU.add)
                        TT(zv[:, :, :, 1, :], zv[:, :, :, 1, :],
                           yv2[:, :, :, 1, 1, :], ALU.add)
                        # sqrt(x + eps): l=0 from ysq n=0; l>=1 in place
                        nc.scalar.activation(zv[:, :, :, 0, :],
                                             yv[:, :, :, 0, :], AF.Sqrt,
                                             bias=eps_t[:])
                        nc.scalar.activation(zv[:, :, :, 1:4, :],
                                             zv[:, :, :, 1:4, :], AF.Sqrt,
                                             bias=eps_t[:])

                    # --- MM3: out[pair, (par, i)] --------------------------
                    ps3 = ps3p.tile([128, 2 * COUT], F32, tag="ps3")
                    for par in range(2 if 'mm3' not in _SKIP else 0):
                        for rl in range(8):
                            lhsT = zv[64 * par:64 * par + 64, :, :, rl // 2,
                                      rl % 2]
                            rhs = w_t[64 * par:64 * par + 64,
                                      COUT * rl:COUT * (rl + 1)]
                            nc.tensor.matmul(
                                ps3[:, COUT * par:COUT * (par + 1)],
                                lhsT, rhs, start=(rl == 0), stop=(rl == 7),
                                skip_group_check=True)

                    # --- relu (+bias) + store ------------------------------
                    osb = osbp.tile([128, 2 * COUT], F32, tag="osb")
                    if with_bias:
                        for par in range(2):
                            nc.vector.tensor_add(
                                osb[:, COUT * par:COUT * (par + 1)],
                                ps3[:, COUT * par:COUT * (par + 1)],
                                bias_t[:, :].broadcast(0, 128))
                        nc.scalar.activation(osb[:], osb[:], AF.Relu)
                    else:
                        nc.vector.tensor_scalar_max(osb[:], ps3[:], 0.0)
                    if 'store' not in _SKIP:
                        dst = outd.ap()[sgb + 256 * h:sgb + 256 * (h + 1), :]
                        dst = dst.rearrange("(pl par) i -> pl par i", par=2)
                        nc.sync.dma_start(
                            dst, osb[:, :].rearrange("p (par i) -> p par i",
                                                     par=2))

    nc.compile()
    return nc


# gather row i -> (v-offset-in-sg, p): part = i%128, chunk = i//128,
# pair q = 4*(chunk//2) + part//32, v = 2q + chunk%2, p = part%32
_II = np.arange(VSG * P)
_VOFF = 2 * (4 * ((_II // 128) // 2) + (_II % 128) // 32) + (_II // 128) % 2
_POFF = (_II % 128) % 32


def _prep_inputs_core(b, signal, patches_idx, conv_kernel, kernel_weights,
                      biases):
    bf = mybir.dt.np(BF16)
    sigrow = np.zeros((V, 128), dtype=bf)
    sigrow[:, 0:CIN] = signal[b].astype(bf)
    sig_u32 = np.ascontiguousarray(sigrow).view(np.uint32)   # [V, 64]

    pidx = patches_idx[b, :, :, 1]
    krn = conv_kernel[b].transpose(0, 1, 3, 2).reshape(V, P, 2 * NSH)
    # kre[sg, 32j+p, 64t + 32b + nr]: b=0: K[vA], b=1: K[vB]-K[vA],
    # with q = 4t+j, vA = sg*512 + 2q, vB = vA + 1
    ka = krn[0::2].reshape(NSG, NSLAB * 2, P, 2 * NSH)       # [sg, q, p, nr]
    kd = (krn[1::2] - krn[0::2]).reshape(NSG, NSLAB * 2, P, 2 * NSH)
    kab = np.stack([ka, kd], axis=3)          # [sg, q, p, b, nr]
    k6 = kab.reshape(NSG, 64, 4, P, 2, 2 * NSH)   # [sg, t, j, p, b, nr]
    kre_ = np.ascontiguousarray(k6.transpose(0, 2, 3, 1, 4, 5)).reshape(
        NSG, 128, NSLAB, 2 * NSH).astype(bf)

    # idx: one op per sg, 16384 idxs; arr[i] = pidx[v(i), p(i)]
    # (rows 16..127 are zero padding -- the gather ucode reads rows 0:16;
    # loading a full-height tensor avoids an uninitialized-SBUF memset)
    idxh = np.zeros((128, V * P // 16), dtype=np.int16)
    ncols = (V * P // 16) // NSG
    for sg in range(NSG):
        arr = pidx[VSG * sg + _VOFF, _POFF].astype(np.int16)
        idxh[:16, ncols * sg:ncols * (sg + 1)] = arr.reshape(ncols, 16).T

    w = kernel_weights.transpose(1, 3, 2, 0).reshape(CIN, 8 * COUT)
    wsb = np.concatenate([w, w], axis=0).astype(bf)
    bia = biases.reshape(1, COUT).astype(np.float32)
    return {"sig": sig_u32, "kre": kre_, "idx": idxh, "wsb": wsb, "bia": bia}


def _make_runner(nc, n_cores=8):
    import jax
    from jax.sharding import Mesh, PartitionSpec
    from jax.experimental.shard_map import shard_map

    bass2jax.install_neuronx_cc_hook()
    partition_name = (nc.partition_id_tensor.name
                      if nc.partition_id_tensor else None)
    in_names, out_names, out_avals, zero_outs = [], [], [], []
    for alloc in nc.m.functions[0].allocations:
        if not isinstance(alloc, mybir.MemoryLocationSet):
            continue
        name = alloc.memorylocations[0].name
        if alloc.kind == "ExternalInput":
            if name != partition_name:
                in_names.append(name)
        elif alloc.kind == "ExternalOutput":
            out_names.append(name)
            shape = tuple(alloc.tensor_shape)
            dtype = mybir.dt.np(alloc.dtype)
            out_avals.append(jax.core.ShapedArray(shape, dtype))
            zero_outs.append(np.zeros(shape, dtype))
    n_params, n_outs = len(in_names), len(out_avals)
    in_names_all = list(in_names) + list(out_names)
    if partition_name is not None:
        in_names_all.append(partition_name)

    def _body(*args):
        operands = list(args)
        if partition_name is not None:
            operands.append(bass2jax.partition_id_tensor())
        outs = bass2jax._bass_exec_p.bind(
            *operands, out_avals=tuple(out_avals),
            in_names=tuple(in_names_all), out_names=tuple(out_names),
            lowering_input_output_aliases=(),
            sim_require_finite=True, sim_require_nnan=True, nc=nc)
        return tuple(outs)

    donate = tuple(range(n_params, n_params + n_outs))
    devices = jax.devices()[:n_cores]
    mesh = Mesh(np.asarray(devices), ("core",))
    sharded = jax.jit(
        shard_map(_body, mesh=mesh,
                  in_specs=(PartitionSpec("core"),) * (n_params + n_outs),
                  out_specs=(PartitionSpec("core"),) * n_outs,
                  check_rep=False),
        donate_argnums=donate, keep_unused=True)

    def run_fn(in_maps):
        import jax
        per_core = [[np.asarray(m[nm]) for nm in in_names] for m in in_maps]
        concat_in = [
            np.concatenate([per_core[c][i] for c in range(n_cores)], axis=0)
            for i in range(n_params)]
        concat_zeros = [
            np.zeros((n_cores * z.shape[0], *z.shape[1:]), z.dtype)
            for z in zero_outs]
        out_arrs = sharded(*concat_in, *concat_zeros)
        jax.block_until_ready(out_arrs)
        return [
            {nm: np.asarray(out_arrs[i]).reshape(n_cores, *out_avals[i].shape)[c]
             for i, nm in enumerate(out_names)}
            for c in range(n_cores)]

    return run_fn


def kernel(signal, patches_idx, conv_kernel, kernel_weights, biases):
    with_bias = bool(np.any(biases))
    key = ("k", with_bias)
    if key not in _CACHE:
        nc = _build_nc(with_bias)
        _CACHE[key] = (nc, _make_runner(nc))
    nc, run = _CACHE[key]

    in_maps = []
    for b in range(B):
        m = _prep_inputs_core(b, signal, patches_idx, conv_kernel,
                              kernel_weights, biases)
        in_maps.append(m)

    results = run(in_maps)
    out = np.stack([results[b]["outd"] for b in range(B)], axis=0)
    return out.astype(np.float32)
